# revision 8
# baseline (speedup 1.0000x reference)
"""Trainium2 Bass kernel for NeuralBlochRK4 (v2: grouped-ACT pipeline).

Reference: RK4 integration (255 steps) of dy/dt = MLP([y,u(t),p,t]) with
MLP 13 -> 128(tanh) -> 128(tanh) -> 3, batch 16384, output (B, 256, 3).

v2 strategy (vs the v1 two-thread emission that left the scalar engine at
51% and the tensor engine stuck cold at 1.2 GHz):
  * Two batch lanes of 1024 per core. Per RK4 stage, the ACT stream is
    [tanh1_A, tanh1_B, tanh2_A, tanh2_B] so every ACT op's PE dependency
    (z2 = W2@h1, or next-stage z1 = Wc@x + alpha*C@h2) finishes two ACT
    ops (~2.3us) before it is needed -- ACT runs back-to-back.
  * PE order per stage: [z2_A, z2_B, z1wc_A', z1wc_B', z1c_A', z1c_B',
    w3_A, w3_B] -- weight-grouped (4 LDWs per stage), and the next
    stage's z1 prefires during the current tanh2 pair.
  * PSUM (8 banks): one 2-bank z set per lane (z1 and z2 share banks --
    the reuse deps coincide with the true tanh deps), one 2-bank
    (3,1024) yp accumulator per lane for the four W3 matmuls.
  * Step boundary: one DVE add per lane folds y_n into yp to produce the
    next x tile's y rows (exact fp32), then DMAs the step output.
  * All matmuls f32r (single-pass PE mode), PSUM accumulation fp32.
    Numerics identical to v1 (same folded constants).
"""

import numpy as np
from contextlib import ExitStack

import concourse.bass as bass
import concourse.tile as tile
from concourse import bacc, mybir
from concourse.bass_utils import run_bass_kernel_spmd

F32 = mybir.dt.float32
F32R = mybir.dt.float32r
TANH = mybir.ActivationFunctionType.Tanh

B_FULL, T_FULL, HID = 16384, 256, 128
N_CORES = 8
NLANES = 2


# ----------------------------------------------------------------------------
# host-side constant preparation (identical folding to v1)
# ----------------------------------------------------------------------------

def prepare_consts(W1, b1, W2, b2, W3, b3, t):
    f32 = np.float32
    W1 = np.asarray(W1, f32); W2 = np.asarray(W2, f32); W3 = np.asarray(W3, f32)
    b1 = np.asarray(b1, f32); b2 = np.asarray(b2, f32); b3 = np.asarray(b3, f32)
    t = np.asarray(t, f32)
    h = f32(t[1] - t[0])

    A = W1[:, 0:3]
    U = W1[:, 3:7]
    P = W1[:, 7:12]
    w_t = W1[:, 12]
    C = (A @ W3).astype(f32)
    Ab3 = (A @ b3).astype(f32)

    stages = [
        (f32(0.0), f32(0.0), f32(1.0), f32(0.0)),
        (f32(h / 2), f32(h / 2), f32(0.5), f32(0.5)),
        (f32(h / 2), f32(h / 2), f32(0.5), f32(0.5)),
        (f32(h), f32(h), f32(0.0), f32(1.0)),
    ]
    Wc = []
    for (o, al, cn, ce) in stages:
        kxm = np.zeros((17, 128), f32)
        kxm[0:3, :] = A.T
        kxm[3, :] = b1 + w_t * o + al * Ab3
        kxm[4:9, :] = P.T
        kxm[9:13, :] = cn * U.T
        kxm[13:17, :] = ce * U.T
        Wc.append(np.ascontiguousarray(kxm))

    consts = {
        "Wc1": Wc[0], "Wc23": Wc[1], "Wc4": Wc[3],
        "Wz": np.zeros((17, 128), f32),
        "Ch": np.ascontiguousarray((f32(h / 2) * C.T).astype(f32)),
        "Cf": np.ascontiguousarray((f32(h) * C.T).astype(f32)),
        "W2T": np.ascontiguousarray(W2.T.astype(f32)),
        "W36": np.ascontiguousarray((f32(h / 6) * W3.T).astype(f32)),
        "W33": np.ascontiguousarray((f32(h / 3) * W3.T).astype(f32)),
        "wtt": np.ascontiguousarray(np.outer(w_t, t).astype(f32)),
        "b2": np.ascontiguousarray(b2.reshape(128, 1)),
        "hb3": np.ascontiguousarray((h * b3).reshape(3, 1)),
    }
    return consts


# ----------------------------------------------------------------------------
# device program
# ----------------------------------------------------------------------------

def build_tile_body(tc, aps, B_core, T, has_b3):
    nc = tc.nc
    W = B_core // NLANES       # per-lane batch width (1024)
    CH = 512                   # matmul free-dim chunk (one PSUM bank, f32)
    NCH = W // CH
    assert W % CH == 0

    with ExitStack() as ctx:
        wpool = ctx.enter_context(tc.tile_pool(name="wts", bufs=1))
        xpool = ctx.enter_context(tc.tile_pool(name="x", bufs=1))
        hpool = ctx.enter_context(tc.tile_pool(name="h", bufs=1))
        zpool = ctx.enter_context(
            tc.tile_pool(name="z", bufs=1, space=bass.MemorySpace.PSUM))
        ypool = ctx.enter_context(
            tc.tile_pool(name="yp", bufs=1, space=bass.MemorySpace.PSUM))

        def wtile(name, shape, dt):
            tl = wpool.tile(list(shape), dt, tag=name)
            nc.sync.dma_start(tl[:, :], aps[name][:, :])
            return tl

        wc1 = wtile("Wc1", (17, 128), F32R)
        wc23 = wtile("Wc23", (17, 128), F32R)
        wc4 = wtile("Wc4", (17, 128), F32R)
        wz = wtile("Wz", (17, 128), F32R)
        ch_t = wtile("Ch", (128, 128), F32R)
        cf_t = wtile("Cf", (128, 128), F32R)
        w2t = wtile("W2T", (128, 128), F32R)
        w36 = wtile("W36", (128, 3), F32R)
        w33 = wtile("W33", (128, 3), F32R)
        wtt = wtile("wtt", (128, T), F32)
        b2t = wtile("b2", (128, 1), F32)
        hb3t = wtile("hb3", (3, 1), F32) if has_b3 else None

        wc_s = (wc1, wc23, wc23, wc4)
        cs_s = (None, ch_t, ch_t, cf_t)   # C weight used by stage s's z1
        w3_s = (w36, w33, w33, w36)

        yout = aps["yout"]      # (3, T-1, B_core) f32r
        uT = aps["uT"]          # (T*4, B_core)   f32r
        xinit = aps["xinit"]    # (17, B_core)    f32r

        # persistent tiles -------------------------------------------------
        # x tiles: [lane][parity] (17, W): rows 0-2 y, 3 ones, 4-8 p,
        # 9-12 u_n, 13-16 u_{n+1}
        xb = [[None, None] for _ in range(NLANES)]
        for l in range(NLANES):
            for par in range(2):
                tl = xpool.tile([17, W], F32R, tag=f"xb{l}{par}", name=f"xb{l}{par}")
                nc.sync.dma_start(tl[:, :], xinit[:, l * W:(l + 1) * W])
                xb[l][par] = tl
        for l in range(NLANES):
            nc.sync.dma_start(xb[l][0][9:17, :], uT[0:8, l * W:(l + 1) * W])
            if T - 1 > 1:
                nc.sync.dma_start(xb[l][1][9:17, :], uT[4:12, l * W:(l + 1) * W])

        h1 = [hpool.tile([128, W], F32R, tag=f"h1_{l}", name=f"h1_{l}")
              for l in range(NLANES)]
        h2 = [hpool.tile([128, W], F32R, tag=f"h2_{l}", name=f"h2_{l}")
              for l in range(NLANES)]

        # PSUM: z set per lane (128, W) = 2 banks; yp per lane (3, W) = 2
        zt = [zpool.tile([128, W], F32, tag=f"z{l}", name=f"z{l}")
              for l in range(NLANES)]
        yp = [ypool.tile([3, W], F32, tag=f"yp{l}", name=f"yp{l}")
              for l in range(NLANES)]

        csl = [slice(c * CH, (c + 1) * CH) for c in range(NCH)]

        # HAM warmers: zero-weight matmuls (add 0 to an open accumulation
        # group) keep the PE activity monitor's busy window satisfied so
        # the clock gate stays at 8/8 during ACT-bound phases.
        NDUM1 = 0   # per lane, inside next-stage z1 group
        NDUM2 = 0   # per lane, inside z2 group
        NDUMB_PRE = 2   # per lane, fills the w3->dve->z1 wait
        DUP_AFTER_W3 = (1, 2)  # stages whose next-z1 gets an idempotent
        # recompute after w3 -- fills PE idle so the HAM clock gate stays
        # at 8/8 without delaying any ACT-feeding op (slack ~1.7us)

        def emit_dummies(l, par, count, start=False, stop=False):
            for d in range(count):
                nc.tensor.matmul(zt[l][:, csl[d % NCH]], wz[:, :],
                                 xb[l][par][:, csl[d % NCH]],
                                 start=(start and d == 0),
                                 stop=(stop and d == count - 1))

        def emit_z1(l, par, s, keep_open=False):
            """z1 for (lane l, stage s): Wc_s@x [+ Cs_s@h2prev]."""
            for sl in csl:
                nc.tensor.matmul(zt[l][:, sl], wc_s[s][:, :],
                                 xb[l][par][:, sl],
                                 start=True,
                                 stop=(s == 0 and not keep_open))

        def emit_z1c(l, s):
            for sl in csl:
                nc.tensor.matmul(zt[l][:, sl], cs_s[s][:, :],
                                 h2[l][:, sl], start=False, stop=True)

        # prologue: z1 of (step 0, stage 0) for both lanes
        for l in range(NLANES):
            emit_z1(l, 0, 0)

        for n in range(T - 1):
            par, nxt = n % 2, (n + 1) % 2
            for s in range(4):
                if s == 0 and n + 1 <= T - 2:
                    r0 = 4 * (n + 1)
                    for l in range(NLANES):
                        nc.sync.dma_start(xb[l][nxt][9:17, :],
                                          uT[r0:r0 + 8, l * W:(l + 1) * W])

                # ACT: tanh1 pair
                for l in range(NLANES):
                    nc.scalar.activation(h1[l][:, :], zt[l][:, :], TANH,
                                         bias=wtt[:, n:n + 1])
                # PE: z2 pair (same z banks, after tanh1 read)
                for l in range(NLANES):
                    for sl in csl:
                        nc.tensor.matmul(zt[l][:, sl], w2t[:, :],
                                         h1[l][:, sl], start=True,
                                         stop=(NDUM2 == 0))
                for l in range(NLANES):
                    if NDUM2:
                        emit_dummies(l, par, NDUM2, stop=True)
                # ACT: tanh2 pair
                for l in range(NLANES):
                    nc.scalar.activation(h2[l][:, :], zt[l][:, :], TANH,
                                         bias=b2t[:, 0:1])

                if s < 3:
                    # PE: next-stage z1 prefire (Wc pair, then C pair)
                    for l in range(NLANES):
                        emit_z1(l, par, s + 1)
                    for l in range(NLANES):
                        if NDUM1:
                            emit_dummies(l, par, NDUM1)
                    for l in range(NLANES):
                        emit_z1c(l, s + 1)
                    # PE: W3 accumulation for this stage
                    for l in range(NLANES):
                        for sl in csl:
                            nc.tensor.matmul(yp[l][0:3, sl], w3_s[s][:, :],
                                             h2[l][:, sl],
                                             start=(s == 0), stop=False)
                    if s in DUP_AFTER_W3:
                        # idempotent z1 recompute (re-clears, same values)
                        for l in range(NLANES):
                            emit_z1(l, par, s + 1)
                        for l in range(NLANES):
                            emit_z1c(l, s + 1)
                else:
                    # boundary: W3 stop first (feeds DVE), then next step z1
                    for l in range(NLANES):
                        for sl in csl:
                            nc.tensor.matmul(yp[l][0:3, sl], w3_s[s][:, :],
                                             h2[l][:, sl],
                                             start=False, stop=True)
                    # DVE: y_{n+1} = yp + y_n (exact fp32), then output DMA
                    for l in range(NLANES):
                        nc.vector.tensor_add(xb[l][nxt][0:3, :], yp[l][:, :],
                                             xb[l][par][0:3, :])
                        if has_b3:
                            nc.vector.tensor_scalar_add(xb[l][nxt][0:3, :],
                                                        xb[l][nxt][0:3, :],
                                                        hb3t[:, 0:1])
                        nc.sync.dma_start(yout[:, n, l * W:(l + 1) * W],
                                          xb[l][nxt][0:3, :])
                    if NDUMB_PRE:
                        for l in range(NLANES):
                            emit_dummies(l, par, NDUMB_PRE, start=True,
                                         stop=True)
                    if n + 1 <= T - 2:
                        for l in range(NLANES):
                            emit_z1(l, nxt, 0)


def build_program(B_core, T, has_b3=False, debug=False, enable_asserts=False):
    nc = bacc.Bacc("TRN2", target_bir_lowering=False, debug=debug,
                   enable_asserts=enable_asserts, num_devices=1)
    shapes = {
        "xinit": ((17, B_core), F32R),
        "uT": ((T * 4, B_core), F32R),
        "Wc1": ((17, 128), F32R), "Wc23": ((17, 128), F32R),
        "Wc4": ((17, 128), F32R), "Wz": ((17, 128), F32R),
        "Ch": ((128, 128), F32R), "Cf": ((128, 128), F32R),
        "W2T": ((128, 128), F32R),
        "W36": ((128, 3), F32R), "W33": ((128, 3), F32R),
        "wtt": ((128, T), F32), "b2": ((128, 1), F32),
    }
    if has_b3:
        shapes["hb3"] = ((3, 1), F32)
    aps = {}
    for name, (shp, dt) in shapes.items():
        aps[name] = nc.dram_tensor(name, list(shp), dt,
                                   kind="ExternalInput").ap()
    aps["yout"] = nc.dram_tensor("yout", [3, T - 1, B_core], F32R,
                                 kind="ExternalOutput").ap()
    with tile.TileContext(nc) as tc:
        build_tile_body(tc, aps, B_core, T, has_b3)
    nc.compile()
    return nc


def make_in_maps(y0, t, u, p, W1, b1, W2, b2, W3, b3, n_cores, B_core, T,
                 has_b3):
    f32 = np.float32
    y0 = np.asarray(y0, f32); u = np.asarray(u, f32); p = np.asarray(p, f32)
    consts = prepare_consts(W1, b1, W2, b2, W3, b3, t)
    if not has_b3:
        consts.pop("hb3")
    in_maps = []
    for i in range(n_cores):
        sl = slice(i * B_core, (i + 1) * B_core)
        xinit = np.zeros((17, B_core), f32)
        xinit[0:3] = y0[sl].T
        xinit[3] = 1.0
        xinit[4:9] = p[sl].T
        uT = np.ascontiguousarray(
            u[sl].transpose(1, 2, 0).reshape(T * 4, B_core))
        m = {"xinit": xinit, "uT": uT}
        m.update(consts)
        in_maps.append(m)
    return in_maps


_PROGRAM_CACHE = {}


def _get_program(B_core, T, has_b3):
    key = (B_core, T, has_b3)
    if key not in _PROGRAM_CACHE:
        _PROGRAM_CACHE[key] = build_program(B_core, T, has_b3)
    return _PROGRAM_CACHE[key]


def run_on_cores(inputs, n_cores=N_CORES, NTH=None, trace=False):
    y0 = np.asarray(inputs["y0"], np.float32)
    B = y0.shape[0]
    T = np.asarray(inputs["t"]).shape[0]
    B_core = B // n_cores
    has_b3 = bool(np.any(np.asarray(inputs["b3"]) != 0))
    nc = _get_program(B_core, T, has_b3)
    in_maps = make_in_maps(
        inputs["y0"], inputs["t"], inputs["u"], inputs["p"],
        inputs["W1"], inputs["b1"], inputs["W2"], inputs["b2"],
        inputs["W3"], inputs["b3"], n_cores, B_core, T, has_b3)
    res = run_bass_kernel_spmd(nc, in_maps, list(range(n_cores)), trace=trace)
    out = np.empty((B, T, 3), np.float32)
    for i in range(n_cores):
        sl = slice(i * B_core, (i + 1) * B_core)
        yo = np.asarray(res.results[i]["yout"])        # (3, T-1, B_core)
        out[sl, 1:, :] = yo.transpose(2, 1, 0)
        out[sl, 0, :] = y0[sl]
    return out, res


def kernel(y0, t, u, p, W1, b1, W2, b2, W3, b3):
    out, _ = run_on_cores(
        dict(y0=y0, t=t, u=u, p=p, W1=W1, b1=b1, W2=W2, b2=b2,
             W3=W3, b3=b3),
        n_cores=N_CORES, trace=False)
    return out


# revision 12
# speedup vs baseline: 1.4954x; 1.4954x over previous
"""Trainium2 Bass kernel for NeuralBlochRK4 (v2: grouped-ACT pipeline).

Reference: RK4 integration (255 steps) of dy/dt = MLP([y,u(t),p,t]) with
MLP 13 -> 128(tanh) -> 128(tanh) -> 3, batch 16384, output (B, 256, 3).

v2 strategy (vs the v1 two-thread emission that left the scalar engine at
51% and the tensor engine stuck cold at 1.2 GHz):
  * Two batch lanes of 1024 per core. Per RK4 stage, the ACT stream is
    [tanh1_A, tanh1_B, tanh2_A, tanh2_B] so every ACT op's PE dependency
    (z2 = W2@h1, or next-stage z1 = Wc@x + alpha*C@h2) finishes two ACT
    ops (~2.3us) before it is needed -- ACT runs back-to-back.
  * PE order per stage: [z2_A, z2_B, z1wc_A', z1wc_B', z1c_A', z1c_B',
    w3_A, w3_B] -- weight-grouped (4 LDWs per stage), and the next
    stage's z1 prefires during the current tanh2 pair.
  * PSUM (8 banks): one 2-bank z set per lane (z1 and z2 share banks --
    the reuse deps coincide with the true tanh deps), one 2-bank
    (3,1024) yp accumulator per lane for the four W3 matmuls.
  * Step boundary: one DVE add per lane folds y_n into yp to produce the
    next x tile's y rows (exact fp32), then DMAs the step output.
  * All matmuls f32r (single-pass PE mode), PSUM accumulation fp32.
    Numerics identical to v1 (same folded constants).
"""

import numpy as np
from contextlib import ExitStack

import concourse.bass as bass
import concourse.tile as tile
from concourse import bacc, mybir
from concourse.bass_utils import run_bass_kernel_spmd

F32 = mybir.dt.float32
F32R = mybir.dt.float32r
F16 = mybir.dt.float16
TANH = mybir.ActivationFunctionType.Tanh

B_FULL, T_FULL, HID = 16384, 256, 128
N_CORES = 8
NLANES = 2


# ----------------------------------------------------------------------------
# host-side constant preparation (identical folding to v1)
# ----------------------------------------------------------------------------

def prepare_consts(W1, b1, W2, b2, W3, b3, t):
    f32 = np.float32
    W1 = np.asarray(W1, f32); W2 = np.asarray(W2, f32); W3 = np.asarray(W3, f32)
    b1 = np.asarray(b1, f32); b2 = np.asarray(b2, f32); b3 = np.asarray(b3, f32)
    t = np.asarray(t, f32)
    h = f32(t[1] - t[0])

    A = W1[:, 0:3]
    U = W1[:, 3:7]
    P = W1[:, 7:12]
    w_t = W1[:, 12]
    C = (A @ W3).astype(f32)
    Ab3 = (A @ b3).astype(f32)

    stages = [
        (f32(0.0), f32(0.0), f32(1.0), f32(0.0)),
        (f32(h / 2), f32(h / 2), f32(0.5), f32(0.5)),
        (f32(h / 2), f32(h / 2), f32(0.5), f32(0.5)),
        (f32(h), f32(h), f32(0.0), f32(1.0)),
    ]
    Wc = []
    for (o, al, cn, ce) in stages:
        kxm = np.zeros((17, 128), f32)
        kxm[0:3, :] = A.T
        kxm[3, :] = b1 + w_t * o + al * Ab3
        kxm[4:9, :] = P.T
        kxm[9:13, :] = cn * U.T
        kxm[13:17, :] = ce * U.T
        Wc.append(np.ascontiguousarray(kxm))

    f16 = np.float16
    consts = {
        "Wz": np.zeros((17, 128), f16),
        "Wc1": Wc[0].astype(f16), "Wc23": Wc[1].astype(f16),
        "Wc4": Wc[3].astype(f16),
        "Ch": np.ascontiguousarray((f32(h / 2) * C.T).astype(f16)),
        "Cf": np.ascontiguousarray((f32(h) * C.T).astype(f16)),
        "W2T": np.ascontiguousarray(W2.T.astype(f16)),
        "W36": np.ascontiguousarray((f32(h / 6) * W3.T).astype(f16)),
        "W33": np.ascontiguousarray((f32(h / 3) * W3.T).astype(f16)),
        "wtt": np.ascontiguousarray(np.outer(w_t, t).astype(f32)),
        "b2": np.ascontiguousarray(b2.reshape(128, 1)),
        "hb3": np.ascontiguousarray((h * b3).reshape(3, 1)),
    }
    return consts


# ----------------------------------------------------------------------------
# device program
# ----------------------------------------------------------------------------

def build_tile_body(tc, aps, B_core, T, has_b3):
    nc = tc.nc
    W = B_core // NLANES       # per-lane batch width (1024)
    CH = 512                   # matmul free-dim chunk (one PSUM bank, f32)
    NCH = W // CH
    assert W % CH == 0

    with ExitStack() as ctx:
        wpool = ctx.enter_context(tc.tile_pool(name="wts", bufs=1))
        xpool = ctx.enter_context(tc.tile_pool(name="x", bufs=1))
        hpool = ctx.enter_context(tc.tile_pool(name="h", bufs=1))
        zpool = ctx.enter_context(
            tc.tile_pool(name="z", bufs=1, space=bass.MemorySpace.PSUM))
        ypool = ctx.enter_context(
            tc.tile_pool(name="yp", bufs=1, space=bass.MemorySpace.PSUM))

        def wtile(name, shape, dt):
            tl = wpool.tile(list(shape), dt, tag=name)
            nc.sync.dma_start(tl[:, :], aps[name][:, :])
            return tl

        wc1 = wtile("Wc1", (17, 128), F16)
        wc23 = wtile("Wc23", (17, 128), F16)
        wc4 = wtile("Wc4", (17, 128), F16)
        wz = wtile("Wz", (17, 128), F16)
        ch_t = wtile("Ch", (128, 128), F16)
        cf_t = wtile("Cf", (128, 128), F16)
        w2t = wtile("W2T", (128, 128), F16)
        w36 = wtile("W36", (128, 3), F16)
        w33 = wtile("W33", (128, 3), F16)
        wtt = wtile("wtt", (128, T), F32)
        b2t = wtile("b2", (128, 1), F32)
        hb3t = wtile("hb3", (3, 1), F32) if has_b3 else None

        wc_s = (wc1, wc23, wc23, wc4)
        cs_s = (None, ch_t, ch_t, cf_t)   # C weight used by stage s's z1
        w3_s = (w36, w33, w33, w36)

        yout = aps["yout"]      # (3, T-1, B_core) f32r
        uT = aps["uT"]          # (T*4, B_core)   f32r
        xinit = aps["xinit"]    # (17, B_core)    f32r

        # persistent tiles -------------------------------------------------
        # x tiles: [lane][parity] (17, W): rows 0-2 y, 3 ones, 4-8 p,
        # 9-12 u_n, 13-16 u_{n+1}
        yinit = aps["yinit"]    # (3, B_core) f32
        xb = [[None, None] for _ in range(NLANES)]
        ym = [[None, None] for _ in range(NLANES)]
        for l in range(NLANES):
            for par in range(2):
                tl = xpool.tile([17, W], F16, tag=f"xb{l}{par}", name=f"xb{l}{par}")
                nc.sync.dma_start(tl[:, :], xinit[:, l * W:(l + 1) * W])
                xb[l][par] = tl
                ytl = xpool.tile([3, W], F32, tag=f"ym{l}{par}", name=f"ym{l}{par}")
                nc.sync.dma_start(ytl[:, :], yinit[:, l * W:(l + 1) * W])
                ym[l][par] = ytl
        for l in range(NLANES):
            nc.sync.dma_start(xb[l][0][9:17, :], uT[0:8, l * W:(l + 1) * W])
            if T - 1 > 1:
                nc.sync.dma_start(xb[l][1][9:17, :], uT[4:12, l * W:(l + 1) * W])

        h1 = [hpool.tile([128, W], F16, tag=f"h1_{l}", name=f"h1_{l}")
              for l in range(NLANES)]
        h2 = [hpool.tile([128, W], F16, tag=f"h2_{l}", name=f"h2_{l}")
              for l in range(NLANES)]

        # PSUM: z set per lane (128, W) = 2 banks; yp per lane (3, W) = 2
        zt = [zpool.tile([128, W], F32, tag=f"z{l}", name=f"z{l}")
              for l in range(NLANES)]
        yp = [ypool.tile([3, W], F32, tag=f"yp{l}", name=f"yp{l}")
              for l in range(NLANES)]

        csl = [slice(c * CH, (c + 1) * CH) for c in range(NCH)]

        # Boundary warmers: the only window where PE has nothing queued is
        # w3(stop) -> DVE -> z1(next step); a ~1.5us contiguous PE idle
        # there re-throttles the HAM clock gate every step (one cold
        # episode per step in the ham trace). Fill it with a right-sized
        # burst of zero-weight matmuls into the dead z banks.
        NDUMB = 2   # per lane

        def emit_dummies(l, par, count):
            for d in range(count):
                nc.tensor.matmul(zt[l][:, csl[d % NCH]], wz[:, :],
                                 xb[l][par][:, csl[d % NCH]],
                                 start=(d == 0), stop=(d == count - 1))


        def emit_z1(l, par, s, keep_open=False):
            """z1 for (lane l, stage s): Wc_s@x [+ Cs_s@h2prev]."""
            for sl in csl:
                nc.tensor.matmul(zt[l][:, sl], wc_s[s][:, :],
                                 xb[l][par][:, sl],
                                 start=True,
                                 stop=(s == 0 and not keep_open))

        def emit_z1c(l, s):
            for sl in csl:
                nc.tensor.matmul(zt[l][:, sl], cs_s[s][:, :],
                                 h2[l][:, sl], start=False, stop=True)

        # prologue: z1 of (step 0, stage 0) for both lanes
        for l in range(NLANES):
            emit_z1(l, 0, 0)

        for n in range(T - 1):
            par, nxt = n % 2, (n + 1) % 2
            for s in range(4):
                if s == 0 and n + 1 <= T - 2:
                    r0 = 4 * (n + 1)
                    for l in range(NLANES):
                        nc.sync.dma_start(xb[l][nxt][9:17, :],
                                          uT[r0:r0 + 8, l * W:(l + 1) * W])

                # ACT: tanh1 pair
                for l in range(NLANES):
                    nc.scalar.activation(h1[l][:, :], zt[l][:, :], TANH,
                                         bias=wtt[:, n:n + 1])
                # PE: z2 pair (same z banks, after tanh1 read)
                for l in range(NLANES):
                    for sl in csl:
                        nc.tensor.matmul(zt[l][:, sl], w2t[:, :],
                                         h1[l][:, sl], start=True,
                                         stop=True)
                # ACT: tanh2 pair
                for l in range(NLANES):
                    nc.scalar.activation(h2[l][:, :], zt[l][:, :], TANH,
                                         bias=b2t[:, 0:1])

                if s < 3:
                    # PE: next-stage z1 prefire (Wc pair, then C pair)
                    for l in range(NLANES):
                        emit_z1(l, par, s + 1)
                    for l in range(NLANES):
                        emit_z1c(l, s + 1)
                    # PE: W3 accumulation for this stage
                    for l in range(NLANES):
                        for sl in csl:
                            nc.tensor.matmul(yp[l][0:3, sl], w3_s[s][:, :],
                                             h2[l][:, sl],
                                             start=(s == 0), stop=False)
                else:
                    # boundary: W3 stop first (feeds DVE), then next step z1
                    for l in range(NLANES):
                        for sl in csl:
                            nc.tensor.matmul(yp[l][0:3, sl], w3_s[s][:, :],
                                             h2[l][:, sl],
                                             start=False, stop=True)
                    # DVE: y_{n+1} = yp + y_n. fp16 copy into the next x
                    # tile (critical path), exact fp32 master for output and
                    # the next step's y (no fp16 error accumulation).
                    for l in range(NLANES):
                        nc.vector.tensor_add(xb[l][nxt][0:3, :], yp[l][:, :],
                                             ym[l][par][:, :])
                        if has_b3:
                            nc.vector.tensor_scalar_add(xb[l][nxt][0:3, :],
                                                        xb[l][nxt][0:3, :],
                                                        hb3t[:, 0:1])
                    for l in range(NLANES):
                        nc.vector.tensor_add(ym[l][nxt][:, :], yp[l][:, :],
                                             ym[l][par][:, :])
                        if has_b3:
                            nc.vector.tensor_scalar_add(ym[l][nxt][:, :],
                                                        ym[l][nxt][:, :],
                                                        hb3t[:, 0:1])
                        nc.sync.dma_start(yout[:, n, l * W:(l + 1) * W],
                                          ym[l][nxt][:, :])
                    if NDUMB:
                        for l in range(NLANES):
                            emit_dummies(l, par, NDUMB)
                    if n + 1 <= T - 2:
                        for l in range(NLANES):
                            emit_z1(l, nxt, 0)


def build_program(B_core, T, has_b3=False, debug=False, enable_asserts=False):
    nc = bacc.Bacc("TRN2", target_bir_lowering=False, debug=debug,
                   enable_asserts=enable_asserts, num_devices=1)
    shapes = {
        "xinit": ((17, B_core), F16),
        "yinit": ((3, B_core), F32),
        "uT": ((T * 4, B_core), F16),
        "Wc1": ((17, 128), F16), "Wc23": ((17, 128), F16),
        "Wc4": ((17, 128), F16), "Wz": ((17, 128), F16),
        "Ch": ((128, 128), F16), "Cf": ((128, 128), F16),
        "W2T": ((128, 128), F16),
        "W36": ((128, 3), F16), "W33": ((128, 3), F16),
        "wtt": ((128, T), F32), "b2": ((128, 1), F32),
    }
    if has_b3:
        shapes["hb3"] = ((3, 1), F32)
    aps = {}
    for name, (shp, dt) in shapes.items():
        aps[name] = nc.dram_tensor(name, list(shp), dt,
                                   kind="ExternalInput").ap()
    aps["yout"] = nc.dram_tensor("yout", [3, T - 1, B_core], F32,
                                 kind="ExternalOutput").ap()
    with tile.TileContext(nc) as tc:
        build_tile_body(tc, aps, B_core, T, has_b3)
    nc.compile()
    return nc


def make_in_maps(y0, t, u, p, W1, b1, W2, b2, W3, b3, n_cores, B_core, T,
                 has_b3):
    f32 = np.float32
    y0 = np.asarray(y0, f32); u = np.asarray(u, f32); p = np.asarray(p, f32)
    consts = prepare_consts(W1, b1, W2, b2, W3, b3, t)
    if not has_b3:
        consts.pop("hb3")
    in_maps = []
    f16 = np.float16
    for i in range(n_cores):
        sl = slice(i * B_core, (i + 1) * B_core)
        xinit = np.zeros((17, B_core), f16)
        xinit[0:3] = y0[sl].T.astype(f16)
        xinit[3] = 1.0
        xinit[4:9] = p[sl].T.astype(f16)
        uT = np.ascontiguousarray(
            u[sl].transpose(1, 2, 0).reshape(T * 4, B_core)).astype(f16)
        yinit = np.ascontiguousarray(y0[sl].T)
        m = {"xinit": xinit, "uT": uT, "yinit": yinit}
        m.update(consts)
        in_maps.append(m)
    return in_maps


_PROGRAM_CACHE = {}


def _get_program(B_core, T, has_b3):
    key = (B_core, T, has_b3)
    if key not in _PROGRAM_CACHE:
        _PROGRAM_CACHE[key] = build_program(B_core, T, has_b3)
    return _PROGRAM_CACHE[key]


def run_on_cores(inputs, n_cores=N_CORES, NTH=None, trace=False):
    y0 = np.asarray(inputs["y0"], np.float32)
    B = y0.shape[0]
    T = np.asarray(inputs["t"]).shape[0]
    B_core = B // n_cores
    has_b3 = bool(np.any(np.asarray(inputs["b3"]) != 0))
    nc = _get_program(B_core, T, has_b3)
    in_maps = make_in_maps(
        inputs["y0"], inputs["t"], inputs["u"], inputs["p"],
        inputs["W1"], inputs["b1"], inputs["W2"], inputs["b2"],
        inputs["W3"], inputs["b3"], n_cores, B_core, T, has_b3)
    res = run_bass_kernel_spmd(nc, in_maps, list(range(n_cores)), trace=trace)
    out = np.empty((B, T, 3), np.float32)
    for i in range(n_cores):
        sl = slice(i * B_core, (i + 1) * B_core)
        yo = np.asarray(res.results[i]["yout"])        # (3, T-1, B_core)
        out[sl, 1:, :] = yo.transpose(2, 1, 0)
        out[sl, 0, :] = y0[sl]
    return out, res


def kernel(y0, t, u, p, W1, b1, W2, b2, W3, b3):
    out, _ = run_on_cores(
        dict(y0=y0, t=t, u=u, p=p, W1=W1, b1=b1, W2=W2, b2=b2,
             W3=W3, b3=b3),
        n_cores=N_CORES, trace=False)
    return out


# revision 16
# speedup vs baseline: 1.5196x; 1.0162x over previous
"""Trainium2 Bass kernel for NeuralBlochRK4 (v2 pipeline, fp16 datapath).

Reference: RK4 integration (255 steps) of dy/dt = MLP([y,u(t),p,t]) with
MLP 13 -> 128(tanh) -> 128(tanh) -> 3, batch 16384, output (B, 256, 3).

Structure (two batch lanes of 1024 per core, grouped-ACT pipeline):
  * Per RK4 stage the ACT stream is [tanh1_A, tanh1_B, tanh2_A, tanh2_B]:
    every ACT op's PE dependency (z2 = W2@h1, or next-stage
    z1 = Wc@x + alpha*C@h2) completes two ACT ops (~2.3us) before it is
    needed, so the scalar engine runs back-to-back.
  * PE order per stage: [z2_A, z2_B, z1wc_A', z1wc_B', z1c_A', z1c_B',
    w3_A, w3_B] -- weight-grouped, next-stage z1 prefires during tanh2.
  * PSUM (8 banks): one 2-bank z set per lane (z1 and z2 share banks;
    the reuse WAR deps coincide with the true tanh deps), one 2-bank
    (3,1024) yp accumulator per lane for the four W3 matmuls.
  * Step boundary: one DVE add per lane writes the fp16 y rows of the
    next x tile (from yp + fp32 ymaster), a second DVE add maintains the
    exact fp32 ymaster used for the output DMA (no fp16 error
    accumulation in the trajectory).
  * All matmul operands fp16 (measured rms 1.8e-4 vs 1.1e-3 for f32r --
    fp16 input rounding beats f32r's multiply truncation), PSUM fp32.
    fp16 also halves SBUF/weight-bus traffic: the tensor engine on this
    part is power/activity-throttled (HAM K=4/8 a large fraction of the
    time), so total PE activity, not peak rate, is the scarce resource.
"""

import numpy as np
from contextlib import ExitStack

import concourse.bass as bass
import concourse.tile as tile
from concourse import bacc, mybir
from concourse.bass_utils import run_bass_kernel_spmd

F32 = mybir.dt.float32
F32R = mybir.dt.float32r
F16 = mybir.dt.float16
TANH = mybir.ActivationFunctionType.Tanh

B_FULL, T_FULL, HID = 16384, 256, 128
N_CORES = 8
NLANES = 2


# ----------------------------------------------------------------------------
# host-side constant preparation
# ----------------------------------------------------------------------------

def prepare_consts(W1, b1, W2, b2, W3, b3, t):
    f32 = np.float32
    f16 = np.float16
    W1 = np.asarray(W1, f32); W2 = np.asarray(W2, f32); W3 = np.asarray(W3, f32)
    b1 = np.asarray(b1, f32); b2 = np.asarray(b2, f32); b3 = np.asarray(b3, f32)
    t = np.asarray(t, f32)
    h = f32(t[1] - t[0])

    A = W1[:, 0:3]
    U = W1[:, 3:7]
    P = W1[:, 7:12]
    w_t = W1[:, 12]
    C = (A @ W3).astype(f32)
    Ab3 = (A @ b3).astype(f32)

    stages = [
        (f32(0.0), f32(0.0), f32(1.0), f32(0.0)),
        (f32(h / 2), f32(h / 2), f32(0.5), f32(0.5)),
        (f32(h / 2), f32(h / 2), f32(0.5), f32(0.5)),
        (f32(h), f32(h), f32(0.0), f32(1.0)),
    ]
    # x-tile rows: [y(3); ones(1); p(5); u_n(4); u_{n+1}(4)]
    Wc = []
    for (o, al, cn, ce) in stages:
        kxm = np.zeros((17, 128), f32)
        kxm[0:3, :] = A.T
        kxm[3, :] = b1 + w_t * o + al * Ab3
        kxm[4:9, :] = P.T
        kxm[9:13, :] = cn * U.T
        kxm[13:17, :] = ce * U.T
        Wc.append(np.ascontiguousarray(kxm))

    consts = {
        "Wc1": Wc[0].astype(f16), "Wc23": Wc[1].astype(f16),
        "Wc4": Wc[3].astype(f16),
        "Ch": np.ascontiguousarray((f32(h / 2) * C.T).astype(f16)),
        "Cf": np.ascontiguousarray((f32(h) * C.T).astype(f16)),
        "W2T": np.ascontiguousarray(W2.T.astype(f16)),
        "W36": np.ascontiguousarray((f32(h / 6) * W3.T).astype(f16)),
        "W33": np.ascontiguousarray((f32(h / 3) * W3.T).astype(f16)),
        "wtt": np.ascontiguousarray(np.outer(w_t, t).astype(f32)),
        "b2": np.ascontiguousarray(b2.reshape(128, 1)),
        "hb3": np.ascontiguousarray((h * b3).reshape(3, 1)),
    }
    return consts


# ----------------------------------------------------------------------------
# device program
# ----------------------------------------------------------------------------

def build_tile_body(tc, aps, B_core, T, has_b3):
    nc = tc.nc
    W = B_core // NLANES       # per-lane batch width (1024)
    CH = 512                   # matmul free-dim chunk (one PSUM bank, f32)
    NCH = W // CH
    assert W % CH == 0

    with ExitStack() as ctx:
        wpool = ctx.enter_context(tc.tile_pool(name="wts", bufs=1))
        xpool = ctx.enter_context(tc.tile_pool(name="x", bufs=1))
        hpool = ctx.enter_context(tc.tile_pool(name="h", bufs=1))
        zpool = ctx.enter_context(
            tc.tile_pool(name="z", bufs=1, space=bass.MemorySpace.PSUM))
        ypool = ctx.enter_context(
            tc.tile_pool(name="yp", bufs=1, space=bass.MemorySpace.PSUM))

        def wtile(name, shape, dt):
            tl = wpool.tile(list(shape), dt, tag=name)
            nc.sync.dma_start(tl[:, :], aps[name][:, :])
            return tl

        wc1 = wtile("Wc1", (17, 128), F16)
        wc23 = wtile("Wc23", (17, 128), F16)
        wc4 = wtile("Wc4", (17, 128), F16)
        ch_t = wtile("Ch", (128, 128), F16)
        cf_t = wtile("Cf", (128, 128), F16)
        w2t = wtile("W2T", (128, 128), F16)
        w36 = wtile("W36", (128, 3), F16)
        w33 = wtile("W33", (128, 3), F16)
        wtt = wtile("wtt", (128, T), F32)
        b2t = wtile("b2", (128, 1), F32)
        hb3t = wtile("hb3", (3, 1), F32) if has_b3 else None

        wc_s = (wc1, wc23, wc23, wc4)
        cs_s = (None, ch_t, ch_t, cf_t)   # C weight used by stage s's z1
        w3_s = (w36, w33, w33, w36)

        yout = aps["yout"]      # (3, T-1, B_core) f32r
        uT = aps["uT"]          # (T*4, B_core)   f16
        xinit = aps["xinit"]    # (17, B_core)    f16
        yinit = aps["yinit"]    # (3, B_core)     f32r

        # persistent tiles -------------------------------------------------
        xb = [[None, None] for _ in range(NLANES)]
        ym = [[None, None] for _ in range(NLANES)]
        for l in range(NLANES):
            for par in range(2):
                tl = xpool.tile([17, W], F16, tag=f"xb{l}{par}",
                                name=f"xb{l}{par}")
                nc.sync.dma_start(tl[:, :], xinit[:, l * W:(l + 1) * W])
                xb[l][par] = tl
                ytl = xpool.tile([3, W], F32R, tag=f"ym{l}{par}",
                                 name=f"ym{l}{par}")
                nc.sync.dma_start(ytl[:, :], yinit[:, l * W:(l + 1) * W])
                ym[l][par] = ytl
        for l in range(NLANES):
            nc.sync.dma_start(xb[l][0][9:17, :], uT[0:8, l * W:(l + 1) * W])
            if T - 1 > 1:
                nc.sync.dma_start(xb[l][1][9:17, :],
                                  uT[4:12, l * W:(l + 1) * W])

        h1 = [hpool.tile([128, W], F16, tag=f"h1_{l}", name=f"h1_{l}")
              for l in range(NLANES)]
        h2 = [hpool.tile([128, W], F16, tag=f"h2_{l}", name=f"h2_{l}")
              for l in range(NLANES)]

        # PSUM: z set per lane (128, W) = 2 banks; yp per lane (3, W) = 2
        zt = [zpool.tile([128, W], F32, tag=f"z{l}", name=f"z{l}")
              for l in range(NLANES)]
        yp = [ypool.tile([3, W], F32, tag=f"yp{l}", name=f"yp{l}")
              for l in range(NLANES)]

        csl = [slice(c * CH, (c + 1) * CH) for c in range(NCH)]

        def emit_z1(l, par, s):
            """z1 for (lane l, stage s): Wc_s@x [+ Cs_s@h2prev]."""
            for sl in csl:
                nc.tensor.matmul(zt[l][:, sl], wc_s[s][:, :],
                                 xb[l][par][:, sl],
                                 start=True, stop=(s == 0))

        def emit_z1c(l, s):
            for sl in csl:
                nc.tensor.matmul(zt[l][:, sl], cs_s[s][:, :],
                                 h2[l][:, sl], start=False, stop=True)

        # prologue: z1 of (step 0, stage 0) for both lanes
        for l in range(NLANES):
            emit_z1(l, 0, 0)

        for n in range(T - 1):
            par, nxt = n % 2, (n + 1) % 2
            for s in range(4):
                if s == 0 and n + 1 <= T - 2:
                    r0 = 4 * (n + 1)
                    for l in range(NLANES):
                        nc.sync.dma_start(xb[l][nxt][9:17, :],
                                          uT[r0:r0 + 8, l * W:(l + 1) * W])

                # ACT: tanh1 pair
                for l in range(NLANES):
                    nc.scalar.activation(h1[l][:, :], zt[l][:, :], TANH,
                                         bias=wtt[:, n:n + 1])
                # PE: z2 pair (same z banks, after tanh1 read)
                for l in range(NLANES):
                    for sl in csl:
                        nc.tensor.matmul(zt[l][:, sl], w2t[:, :],
                                         h1[l][:, sl], start=True, stop=True)
                # ACT: tanh2 pair
                for l in range(NLANES):
                    nc.scalar.activation(h2[l][:, :], zt[l][:, :], TANH,
                                         bias=b2t[:, 0:1])

                if s < 3:
                    # PE: next-stage z1 prefire (Wc pair, then C pair)
                    for l in range(NLANES):
                        emit_z1(l, par, s + 1)
                    for l in range(NLANES):
                        emit_z1c(l, s + 1)
                    # PE: W3 accumulation for this stage
                    for l in range(NLANES):
                        for sl in csl:
                            nc.tensor.matmul(yp[l][0:3, sl], w3_s[s][:, :],
                                             h2[l][:, sl],
                                             start=(s == 0), stop=False)
                else:
                    # boundary: W3 stop first (feeds DVE), then next step z1
                    for l in range(NLANES):
                        for sl in csl:
                            nc.tensor.matmul(yp[l][0:3, sl], w3_s[s][:, :],
                                             h2[l][:, sl],
                                             start=False, stop=True)
                    # DVE: fp16 y rows for the next x tile (critical path),
                    # then the exact fp32 master for output + next step
                    for l in range(NLANES):
                        nc.vector.tensor_add(xb[l][nxt][0:3, :], yp[l][:, :],
                                             ym[l][par][:, :])
                        if has_b3:
                            nc.vector.tensor_scalar_add(xb[l][nxt][0:3, :],
                                                        xb[l][nxt][0:3, :],
                                                        hb3t[:, 0:1])
                    if n + 1 <= T - 2:
                        for l in range(NLANES):
                            emit_z1(l, nxt, 0)
                    for l in range(NLANES):
                        nc.vector.tensor_add(ym[l][nxt][:, :], yp[l][:, :],
                                             ym[l][par][:, :])
                        if has_b3:
                            nc.vector.tensor_scalar_add(ym[l][nxt][:, :],
                                                        ym[l][nxt][:, :],
                                                        hb3t[:, 0:1])
                        nc.sync.dma_start(yout[:, n, l * W:(l + 1) * W],
                                          ym[l][nxt][:, :])


def build_program(B_core, T, has_b3=False, debug=False, enable_asserts=False):
    nc = bacc.Bacc("TRN2", target_bir_lowering=False, debug=debug,
                   enable_asserts=enable_asserts, num_devices=1)
    shapes = {
        "xinit": ((17, B_core), F16),
        "yinit": ((3, B_core), F32R),
        "uT": ((T * 4, B_core), F16),
        "Wc1": ((17, 128), F16), "Wc23": ((17, 128), F16),
        "Wc4": ((17, 128), F16),
        "Ch": ((128, 128), F16), "Cf": ((128, 128), F16),
        "W2T": ((128, 128), F16),
        "W36": ((128, 3), F16), "W33": ((128, 3), F16),
        "wtt": ((128, T), F32), "b2": ((128, 1), F32),
    }
    if has_b3:
        shapes["hb3"] = ((3, 1), F32)
    aps = {}
    for name, (shp, dt) in shapes.items():
        aps[name] = nc.dram_tensor(name, list(shp), dt,
                                   kind="ExternalInput").ap()
    aps["yout"] = nc.dram_tensor("yout", [3, T - 1, B_core], F32R,
                                 kind="ExternalOutput").ap()
    with tile.TileContext(nc) as tc:
        build_tile_body(tc, aps, B_core, T, has_b3)
    nc.compile()
    return nc


def make_in_maps(y0, t, u, p, W1, b1, W2, b2, W3, b3, n_cores, B_core, T,
                 has_b3):
    f32 = np.float32
    f16 = np.float16
    y0 = np.asarray(y0, f32); u = np.asarray(u, f32); p = np.asarray(p, f32)
    consts = prepare_consts(W1, b1, W2, b2, W3, b3, t)
    if not has_b3:
        consts.pop("hb3")
    in_maps = []
    for i in range(n_cores):
        sl = slice(i * B_core, (i + 1) * B_core)
        xinit = np.zeros((17, B_core), f16)
        xinit[0:3] = y0[sl].T.astype(f16)
        xinit[3] = 1.0
        xinit[4:9] = p[sl].T.astype(f16)
        uT = np.ascontiguousarray(
            u[sl].transpose(1, 2, 0).reshape(T * 4, B_core)).astype(f16)
        yinit = np.ascontiguousarray(y0[sl].T)
        m = {"xinit": xinit, "uT": uT, "yinit": yinit}
        m.update(consts)
        in_maps.append(m)
    return in_maps


_PROGRAM_CACHE = {}


def _get_program(B_core, T, has_b3):
    key = (B_core, T, has_b3)
    if key not in _PROGRAM_CACHE:
        _PROGRAM_CACHE[key] = build_program(B_core, T, has_b3)
    return _PROGRAM_CACHE[key]


def run_on_cores(inputs, n_cores=N_CORES, NTH=None, trace=False):
    y0 = np.asarray(inputs["y0"], np.float32)
    B = y0.shape[0]
    T = np.asarray(inputs["t"]).shape[0]
    B_core = B // n_cores
    has_b3 = bool(np.any(np.asarray(inputs["b3"]) != 0))
    nc = _get_program(B_core, T, has_b3)
    in_maps = make_in_maps(
        inputs["y0"], inputs["t"], inputs["u"], inputs["p"],
        inputs["W1"], inputs["b1"], inputs["W2"], inputs["b2"],
        inputs["W3"], inputs["b3"], n_cores, B_core, T, has_b3)
    res = run_bass_kernel_spmd(nc, in_maps, list(range(n_cores)), trace=trace)
    out = np.empty((B, T, 3), np.float32)
    for i in range(n_cores):
        sl = slice(i * B_core, (i + 1) * B_core)
        yo = np.asarray(res.results[i]["yout"])        # (3, T-1, B_core)
        out[sl, 1:, :] = yo.transpose(2, 1, 0)
        out[sl, 0, :] = y0[sl]
    return out, res


def kernel(y0, t, u, p, W1, b1, W2, b2, W3, b3):
    out, _ = run_on_cores(
        dict(y0=y0, t=t, u=u, p=p, W1=W1, b1=b1, W2=W2, b2=b2,
             W3=W3, b3=b3),
        n_cores=N_CORES, trace=False)
    return out
